# revision 8
# baseline (speedup 1.0000x reference)
"""Trainium2 Bass kernel: 3x3 sliding-window variance (zero-padded, stride 1).

Input  x: (8, 32, 512, 512) float32
Output  : (8, 32, 512, 512) float32,  var = E[x^2] - E[x]^2 over each 3x3
          window (divisor 9 everywhere, zero padding).

Sharding: batch dim across the 8 cores (core i gets x[i], no communication).

v3 design (vs v1 baseline):
  - Loads stay on the HWDGE SP ring (fp32), stores move to fp16 on the ACT
    ring: store traffic halves (total HBM 50 MB vs 67 MB); the host casts the
    fp16 result back to fp32.  Variance <= ~30 here, fp16 rel err ~5e-4 is
    far inside the 2e-2 gate.
  - The squared stream is ACT Square(xraw) fp32->fp16 in ONE pass (the
    activation casts on the way out), replacing the GPSIMD tensor_mul pass
    (GPSIMD is ~4x slower per element than DVE/ACT).  DVE still casts the
    x stream (tensor_copy).
  - Bottom rows (504..511) of all 4 images in a group are packed into ONE
    matmul set via a block-diagonal [64,128] weight matrix (4 x [16,8]
    blocks): 6 matmuls per group instead of 24 (816 total MMs vs 960).
  - PSUM is used as [128,1024] double-bank tiles (2 images per tile) and the
    two evac ops run at 1024 width: fewer, wider ACT/DVE instructions, with
    2 tiles x 2 in flight = all 8 banks (double-buffered).

Per-core engine budget (timeline-sim, 207us total): PE ~180us, DMA ~143us,
ACT ~138us, DVE ~120us, GPSIMD idle.

Matmul structure per 128-row chunk (per image, per stream): banded fp16
weights A (entries 7/64, exact in fp16) encode the vertical 3-tap sum and
H-edge zero padding; horizontal 3 taps are 3 column-shifted matmuls
accumulating into one PSUM region (center first with start=True, then
left/right on shrunken ranges for the W edges).  Final ops rescale by
s = 1/(9*(7/64)) in fp32: ACT evacuates mean^2 = Square(s*psum1), DVE
computes var = (s*psum2) - mean^2 in one scalar_tensor_tensor.
"""

import os

import numpy as np

import concourse.bacc as bacc
import concourse.bass as bass
import concourse.mybir as mybir
import concourse.tile as tile
from concourse.bass_utils import run_bass_kernel_spmd

F32 = mybir.dt.float32
F16 = mybir.dt.float16

B, C, H, W = 8, 32, 512, 512
G = 4          # images side by side in the free dim of a main tile
N_GROUPS = C // G
CHUNK = 126    # output rows per main chunk
N_CHUNKS = 4   # main chunks cover rows 0..503; bottom pack covers 504..511
BOT_K = 16     # bottom input rows per image (496..511)
CW = 7.0 / 64.0           # weight value, exact in fp16
SCALE = 1.0 / (9.0 * CW)  # rescale applied in fp32 at evacuation


def _a_matrices():
    # A_top [127,128]: chunk 0, input row k = image row k; out m = image row
    # m, taps rows m-1, m, m+1 (row -1 is zero padding -> band truncated).
    # M padded to 128 (cols 126..127 zero) to enable fast weight load (FWL).
    a_top = np.zeros((127, 128), np.float32)
    for m in range(126):
        for k in (m - 1, m, m + 1):
            if 0 <= k < 127:
                a_top[k, m] = CW
    # A_mid [128,128]: chunk j>=1, input row k = image row 126j-1+k; out m =
    # image row 126j+m, taps k = m, m+1, m+2 (cols 126..127 zero-padded).
    a_mid = np.zeros((128, 128), np.float32)
    for m in range(126):
        for k in (m, m + 1, m + 2):
            a_mid[k, m] = CW
    # A_bot4 [64,128]: block-diagonal pack of 4 images' bottom strips.
    # For image slot j (0..3): input row k' = 16j + k holds image row 496+k;
    # out col m' = 8j + m is image row 504+m, taps k = 7+m, 8+m, 9+m
    # intersected with k<=15 (row 512 is zero padding).  Cols 32..127 zero
    # padding for FWL (needs 128 weight columns).
    a_bot4 = np.zeros((64, 128), np.float32)
    for j in range(4):
        for m in range(8):
            for k in (7 + m, 8 + m, 9 + m):
                if k <= 15:
                    a_bot4[16 * j + k, 8 * j + m] = CW
    return (a_top.astype(np.float16), a_mid.astype(np.float16),
            a_bot4.astype(np.float16))


def _shifted_mms(nc, ps, a_ap, src, col, pcol, width, m_rows):
    """3 column-shifted accumulating matmuls: horizontal 3-tap box sum.

    center tap first (full width, start=True -> every psum element gets its
    has_written bit), then left/right taps accumulate on shrunken ranges so
    edge columns receive exactly the taps that exist.  pcol = column offset
    inside the (possibly multi-bank) psum tile.
    """
    nc.tensor.matmul(ps[0:m_rows, pcol:pcol + width], a_ap,
                     src[:, col:col + width], start=True, stop=False)
    # left tap: out w reads in w-1 => out cols 1.. from in cols 0..width-2
    nc.tensor.matmul(ps[0:m_rows, pcol + 1:pcol + width], a_ap,
                     src[:, col:col + width - 1], start=False, stop=False)
    # right tap: out w reads in w+1 => out cols 0..width-2 from in cols 1..
    nc.tensor.matmul(ps[0:m_rows, pcol:pcol + width - 1], a_ap,
                     src[:, col + 1:col + width], start=False, stop=True)


def build_program():
    nc = bacc.Bacc("TRN2", target_bir_lowering=False, debug=False)
    x = nc.declare_dram_parameter("x", [C, H, W], F32, isOutput=False)
    a_top = nc.declare_dram_parameter("a_top", [127, 128], F16, isOutput=False)
    a_mid = nc.declare_dram_parameter("a_mid", [128, 128], F16, isOutput=False)
    a_bot = nc.declare_dram_parameter("a_bot", [64, 128], F16, isOutput=False)
    y = nc.declare_dram_parameter("y", [C, H, W], F16, isOutput=True)

    with tile.TileContext(nc) as tc:
        with (
            tc.tile_pool(name="const", bufs=1) as cpool,
            tc.tile_pool(name="xraw", bufs=4) as rawpool,
            tc.tile_pool(name="x16p", bufs=4) as x16pool,
            tc.tile_pool(name="sqp", bufs=4) as sqpool,
            tc.tile_pool(name="mmp", bufs=4) as mmpool,
            tc.tile_pool(name="outp", bufs=3) as outpool,
            tc.tile_pool(name="botin", bufs=2) as botinpool,
            tc.tile_pool(name="botout", bufs=2) as botoutpool,
            tc.tile_pool(name="ps", bufs=4, space="PSUM") as pspool,
        ):
            at = cpool.tile([127, 128], F16, tag="at")
            am = cpool.tile([128, 128], F16, tag="am")
            ab = cpool.tile([64, 128], F16, tag="ab")
            nc.sync.dma_start(at[:], a_top[:])
            nc.sync.dma_start(am[:], a_mid[:])
            nc.sync.dma_start(ab[:], a_bot[:])

            def emit_body():
                _emit_body(nc, x, y, at, am, ab, rawpool, x16pool, sqpool,
                           mmpool, outpool, botinpool, botoutpool, pspool)

            repeat = int(os.environ.get("CHVAR_BENCH_REPEAT", "0"))
            if repeat > 1:
                with tc.For_i(0, repeat, 1):
                    emit_body()
            else:
                emit_body()
    return nc


def _emit_main_chunk(nc, x, y, a_ap, rows, r0, yr0, i0, rawpool,
                     x16pool, sqpool, mmpool, outpool, pspool):
    xraw = rawpool.tile([128, G * W], F32, tag="xraw")
    nc.sync.dma_start(
        xraw[0:rows].rearrange("p (i w) -> p i w", i=G),
        x[i0:i0 + G, r0:r0 + rows, :].rearrange("i p w -> p i w"),
    )
    x16 = x16pool.tile([128, G * W], F16, tag="x16")
    nc.vector.tensor_copy(x16[0:rows, :], xraw[0:rows, :])
    # Square activation casts fp32->fp16 on the way out: no separate
    # cast is needed for the squared stream
    sq = sqpool.tile([128, G * W], F16, tag="sq")
    nc.scalar.activation(sq[0:rows, :], xraw[0:rows, :],
                         mybir.ActivationFunctionType.Square)
    outt = outpool.tile([128, G * W], F16, tag="outt")
    for half in range(G // 2):
        ps1 = pspool.tile([128, 2 * W], F32, tag="ps")
        ps2 = pspool.tile([128, 2 * W], F32, tag="ps")
        for bk in range(2):
            col = W * (2 * half + bk)
            _shifted_mms(nc, ps1, a_ap[0:rows, :], x16[0:rows, :],
                         col, W * bk, W, 128)
            _shifted_mms(nc, ps2, a_ap[0:rows, :], sq[0:rows, :],
                         col, W * bk, W, 128)
        mmt = mmpool.tile([128, 2 * W], F16, tag="mmt")
        ocol = 2 * W * half
        # mean^2 = Square(s * psum1); var = (s * psum2) - mean^2
        nc.scalar.activation(mmt[0:CHUNK, :], ps1[0:CHUNK, :],
                             mybir.ActivationFunctionType.Square, scale=SCALE)
        nc.vector.scalar_tensor_tensor(
            outt[0:CHUNK, ocol:ocol + 2 * W], ps2[0:CHUNK, :], SCALE,
            mmt[0:CHUNK, :], mybir.AluOpType.mult, mybir.AluOpType.subtract)
    nc.scalar.dma_start(
        y[i0:i0 + G, yr0:yr0 + CHUNK, :].rearrange("i p w -> p i w"),
        outt[0:CHUNK].rearrange("p (i w) -> p i w", i=G),
    )


def _emit_bottom(nc, x, y, ab, i0, botinpool, botoutpool, mmpool, pspool):
    # 4 images' rows 496..511 stacked in the partition dim: [64, 512]
    xrawb = botinpool.tile([64, W], F32, tag="xrawb")
    for i in range(G):
        nc.sync.dma_start(
            xrawb[BOT_K * i:BOT_K * (i + 1), :],
            x[i0 + i, H - BOT_K:H, :],
        )
    x16b = botinpool.tile([64, W], F16, tag="x16b")
    nc.vector.tensor_copy(x16b[:], xrawb[:])
    sqb = botinpool.tile([64, W], F16, tag="sqb")
    nc.scalar.activation(sqb[:], xrawb[:],
                         mybir.ActivationFunctionType.Square)
    # both streams share one double-bank psum tile: x in cols 0:512,
    # x^2 in cols 512:1024
    ps = pspool.tile([128, 2 * W], F32, tag="ps")
    _shifted_mms(nc, ps, ab[:, :], x16b[:], 0, 0, W, 128)
    _shifted_mms(nc, ps, ab[:, :], sqb[:], 0, W, W, 128)
    mmtb = mmpool.tile([128, 2 * W], F16, tag="mmt")
    nc.scalar.activation(mmtb[0:32, 0:W], ps[0:32, 0:W],
                         mybir.ActivationFunctionType.Square, scale=SCALE)
    outb = botoutpool.tile([32, W], F16, tag="outb")
    nc.vector.scalar_tensor_tensor(
        outb[:, :], ps[0:32, W:2 * W], SCALE, mmtb[0:32, 0:W],
        mybir.AluOpType.mult, mybir.AluOpType.subtract)
    for i in range(G):
        nc.scalar.dma_start(
            y[i0 + i, H - 8:H, :],
            outb[8 * i:8 * (i + 1), :],
        )


def _emit_body(nc, x, y, at, am, ab, rawpool, x16pool, sqpool, mmpool,
               outpool, botinpool, botoutpool, pspool):
    for grp in range(N_GROUPS):
        i0 = grp * G
        for j in range(N_CHUNKS):
            if j == 0:
                r0, rows, a_ap = 0, 127, at
            else:
                r0, rows, a_ap = CHUNK * j - 1, 128, am
            _emit_main_chunk(nc, x, y, a_ap, rows, r0, CHUNK * j, i0,
                             rawpool, x16pool, sqpool, mmpool, outpool,
                             pspool)
        _emit_bottom(nc, x, y, ab, i0, botinpool, botoutpool, mmpool, pspool)


_CACHE = {}


def _get_program():
    if "nc" not in _CACHE:
        nc = build_program()
        nc.finalize()
        _CACHE["nc"] = nc
    return _CACHE["nc"]


def kernel(x: np.ndarray, _trace: bool = False, **_ignored):
    assert x.shape == (B, C, H, W), x.shape
    x = np.ascontiguousarray(x, np.float32)
    nc = _get_program()
    a_top, a_mid, a_bot = _a_matrices()
    in_maps = [
        {"x": x[i], "a_top": a_top, "a_mid": a_mid, "a_bot": a_bot}
        for i in range(B)
    ]
    res = run_bass_kernel_spmd(nc, in_maps, list(range(B)), trace=_trace)
    out = np.stack([res.results[i]["y"] for i in range(B)], axis=0)
    out = out.astype(np.float32)
    if _trace:
        return out, res
    return out
